# revision 1
# baseline (speedup 1.0000x reference)
"""Trainium2 Bass kernel: Performer (linear) attention + in/out projections.

Problem nn_LinearPerformerAttention_6717328851263:
  x:(4,4096,1024) f32, w_qkv:(1024,3072), proj_matrix:(16,64,256),
  w_out:(1024,1024), b_out:(1024,)

  qkv = x @ w_qkv ; split q,k,v ; per (b,h): q_proj=elu1(q@P_h), k_proj=elu1(k@P_h)
  kv = k_proj^T v ; k_sum = sum_n k_proj ; attn = (q_proj @ kv) / (q_proj@k_sum)
  out = attn @ w_out + b_out

Sharding over 8 cores: core c -> (batch b=c//2, head-group g=c%2: 8 of 16 heads).
Each core computes partial y_c = attn(b, heads_g) @ w_out[512g:512g+512, :].
Host gather: out[b] = y_(b,0) + y_(b,1) + b_out.

Device algorithm per core (all layouts chosen so TensorE contracts on the
partition dim without any on-chip transposes of big tensors; x is transposed
on the host):
  pass A (per 512-token group): qT,kT = Wq^T xT, Wk^T xT ; v = x Wv
    k_projE = elu1(kT_h^T proj_h) ; kvT_h[d,f] += [v_h|1]^T k_projE  (PSUM accum)
    qT spilled to DRAM scratch.
  fixup: kvT -> transpose -> kvS_h [F,65] (cols: kv | k_sum)
  pass B (per 512-token group): q_projT_h = elu1(proj_h^T qT_h)
    attnT(pair-packed [128,512]) = kvS^T q_projT ; denom = k_sum^T q_projT
    z = 1/denom ; attnT *= bcast(z) ; y = attnT^T @ w_out -> DRAM
elu1(x) = elu(x)+1 = min(exp(x),1) + relu(x), computed in 3 elementwise ops.
Matmuls run as float32r (fp32 data, fast PE mode).
"""

import numpy as np
from contextlib import ExitStack

import concourse.bass as bass
import concourse.bacc as bacc
import concourse.tile as tile
from concourse import mybir
from concourse.bass_utils import run_bass_kernel_spmd
from concourse.masks import make_identity

FP32 = mybir.dt.float32
F32R = mybir.dt.float32r
AL = mybir.AluOpType
AF = mybir.ActivationFunctionType

B, SEQ, D = 4, 4096, 1024
H, HD, F = 16, 64, 256
HPC = 8            # heads per core
DH = HPC * HD      # 512 head-space dims per core
P = 128
NCORES = 8


def _emit(tc, n, xT, wq, wk, wv, proj, wout, y, qTd, dbg=None):
    nc = tc.nc
    NG = n // 512       # token groups
    TPG = 4             # 128-token tiles per group

    ctx = ExitStack()
    with ctx:
        const = ctx.enter_context(tc.tile_pool(name="const", bufs=1))

        ident = const.tile([P, P], FP32, tag="ident", name="ident")
        make_identity(nc, ident)
        ones_sb = const.tile([P, P], FP32, tag="ones_sb", name="ones_sb")
        nc.vector.memset(ones_sb, 1.0)

        # proj, pair-packed [128, 256]: head 2i at partitions 0:64, head
        # 2i+1 at 64:128 (so lhsT/rhs partition bases always match).
        proj_pair = [const.tile([P, F], F32R, tag=f"projp{i}", name=f"projp{i}") for i in range(4)]
        for i in range(4):
            nc.sync.dma_start(out=proj_pair[i], in_=proj[i * P:(i + 1) * P, :])

        zeros_sb = const.tile([P, P], FP32, tag="zeros_sb", name="zeros_sb")
        nc.vector.memset(zeros_sb, 0.0)

        # attn lhsT, zero-padded to M=128 so a head pair accumulates into one
        # [128,512] PSUM tile (matmul outputs must start at partition 0):
        # kvS[h][s]: [F-slab 128, 128]; head-half cols (h%2)*64.. hold kv_h,
        # the other 64 cols are zero.
        kvS = [[const.tile([P, P], F32R, tag=f"kvS{h}_{s}", name=f"kvS{h}_{s}")
                for s in range(2)] for h in range(HPC)]
        # same trick for the denominator: ksr[h][s] cols (h%2)*64.. replicate
        # k_sum_h, rest zero -> pair denominators land on the matching
        # partitions of one PSUM tile (z broadcast for free)
        ksr = [[const.tile([P, P], F32R, tag=f"ksr{h}_{s}", name=f"ksr{h}_{s}")
                for s in range(2)] for h in range(HPC)]
        for h in range(HPC):
            ho = HD - (h % 2) * HD
            for s in range(2):
                nc.scalar.copy(kvS[h][s][:, ho:ho + HD], zeros_sb[:, 0:HD])
                nc.scalar.copy(ksr[h][s][:, ho:ho + HD], zeros_sb[:, 0:HD])

        # ---------------- pass A ----------------
        with ExitStack() as actx:
            # kv state accumulators: kvT per head-pair [65, 512] in SBUF
            # (PSUM accumulation held open across groups breaks on HW, so
            # accumulate per group in PSUM and fold into SBUF).
            # cols: head 2i -> 0:256, head 2i+1 -> 256:512.
            # row 64 = k_sum (ones column of vone).
            kvaccp = actx.enter_context(tc.tile_pool(name="kvaccp", bufs=1))
            kv_acc = [kvaccp.tile([HD + 1, 512], FP32, tag=f"kva{i}", name=f"kva{i}")
                      for i in range(4)]
            wpool = actx.enter_context(tc.tile_pool(name="wpool", bufs=1))
            wq_sb = [wpool.tile([P, DH], F32R, tag=f"wq{s}", name=f"wq{s}") for s in range(8)]
            wk_sb = [wpool.tile([P, DH], F32R, tag=f"wk{s}", name=f"wk{s}") for s in range(8)]
            wv_sb = [wpool.tile([P, DH], F32R, tag=f"wv{s}", name=f"wv{s}") for s in range(8)]
            for s in range(8):
                nc.sync.dma_start(out=wq_sb[s], in_=wq[s * P:(s + 1) * P, :])
                nc.sync.dma_start(out=wk_sb[s], in_=wk[s * P:(s + 1) * P, :])
                nc.sync.dma_start(out=wv_sb[s], in_=wv[s * P:(s + 1) * P, :])

            xtpool = actx.enter_context(tc.tile_pool(name="xtpool", bufs=2))
            ktpool = actx.enter_context(tc.tile_pool(name="ktpool", bufs=2))
            qtpool = actx.enter_context(tc.tile_pool(name="qtpool", bufs=3))
            vpool = actx.enter_context(tc.tile_pool(name="vpool", bufs=2))
            elupool = actx.enter_context(tc.tile_pool(name="elupool", bufs=4))
            mmps = actx.enter_context(tc.tile_pool(name="mmps", bufs=2, space="PSUM"))
            kpps = actx.enter_context(tc.tile_pool(name="kpps", bufs=2, space="PSUM"))

            xT_v = xT.rearrange("(s p) m -> p s m", p=P)

            for g in range(NG):
                g0 = g * 512
                xt = xtpool.tile([P, 8, 512], F32R, tag="xt", name="xt")
                nc.sync.dma_start(out=xt, in_=xT_v[:, :, g0:g0 + 512])

                # qT: spill to DRAM scratch
                for fs in range(4):
                    ps = mmps.tile([P, 512], FP32, tag="mm", name="mm")
                    for s in range(8):
                        nc.tensor.matmul(
                            ps, lhsT=(wq_sb[s][:, fs * P:(fs + 1) * P]),
                            rhs=(xt[:, s, :]), start=(s == 0), stop=(s == 7))
                    qt_sb = qtpool.tile([P, 512], F32R, tag="qt", name="qt")
                    nc.any.tensor_copy(qt_sb, ps)
                    nc.scalar.dma_start(
                        out=qTd[fs * P:(fs + 1) * P, g0:g0 + 512], in_=qt_sb)

                # kT: kept in SBUF for this group
                kt_sb = [ktpool.tile([P, 512], F32R, tag=f"kt{fs}", name=f"kt{fs}")
                         for fs in range(4)]
                for fs in range(4):
                    ps = mmps.tile([P, 512], FP32, tag="mm", name="mm")
                    for s in range(8):
                        nc.tensor.matmul(
                            ps, lhsT=(wk_sb[s][:, fs * P:(fs + 1) * P]),
                            rhs=(xt[:, s, :]), start=(s == 0), stop=(s == 7))
                    nc.any.tensor_copy(kt_sb[fs], ps)

                # v with ones column: vone[p, t, h, 0:64]=v, [..,64]=1
                vone = vpool.tile([P, TPG, HPC, HD + 1], F32R, tag="vone", name="vone")
                nc.vector.tensor_copy(
                    vone[:, :, :, HD],
                    ones_sb[:, 0:TPG * HPC].rearrange(
                        "p (t h) -> p t h", t=TPG))
                for t in range(TPG):
                    ps = mmps.tile([P, 512], FP32, tag="mm", name="mm")
                    for s in range(8):
                        nc.tensor.matmul(
                            ps, lhsT=(xt[:, s, t * P:(t + 1) * P]),
                            rhs=(wv_sb[s]), start=(s == 0), stop=(s == 7))
                    nc.any.tensor_copy(
                        vone[:, t, :, 0:HD],
                        ps.rearrange("p (h e) -> p h e", h=HPC))

                # k_proj + elu1 + kv accumulation.  Even/odd head kproj
                # matmuls are emitted back-to-back: their lhsT/rhs sit at
                # base partitions 0/64, so the PE runs them concurrently in
                # disjoint row groups.  kv accumulates in a short-lived PSUM
                # tile per (tp, pair), folded into the SBUF accumulator.
                for hp in range(HPC // 2):
                    for tp in range(2):
                        kv_ps = kpps.tile([HD + 1, 512], FP32, tag="kvg",
                                          name="kvg")
                        cps = {}
                        for h in (2 * hp, 2 * hp + 1):
                            hb = (h % 2) * HD
                            c = kpps.tile([P, 512], FP32, tag=f"kp{h % 2}",
                                          name=f"kp{h % 2}")
                            for ti in range(2):
                                t = tp * 2 + ti
                                nc.tensor.matmul(
                                    c[:, ti * F:(ti + 1) * F],
                                    lhsT=(kt_sb[hp][hb:hb + HD,
                                                    t * P:(t + 1) * P]),
                                    rhs=(proj_pair[hp][hb:hb + HD, :]),
                                    start=True, stop=True)
                            cps[h] = c
                        for h in (2 * hp, 2 * hp + 1):
                            kE = elupool.tile([P, 512], FP32, tag="kE", name="kE")
                            kR = elupool.tile([P, 512], FP32, tag="kR", name="kR")
                            kP = elupool.tile([P, 512], F32R, tag="kP", name="kP")
                            nc.scalar.activation(kE, cps[h], AF.Exp)
                            nc.vector.tensor_scalar_max(kR, cps[h], 0.0)
                            nc.vector.scalar_tensor_tensor(
                                kP, in0=kE, scalar=1.0, in1=kR,
                                op0=AL.min, op1=AL.add)
                            if dbg is not None and g == 0:
                                nc.sync.dma_start(out=dbg["kp"][h, tp],
                                                  in_=kP.bitcast(FP32))
                            for ti in range(2):
                                t = tp * 2 + ti
                                nc.tensor.matmul(
                                    kv_ps[:, (h % 2) * F:(h % 2 + 1) * F],
                                    lhsT=(vone[:, t, h, :]),
                                    rhs=(kP[:, ti * F:(ti + 1) * F]),
                                    start=(ti == 0),
                                    stop=(ti == 1),
                                    skip_group_check=True)
                        if g == 0 and tp == 0:
                            nc.vector.tensor_copy(kv_acc[hp], kv_ps)
                        else:
                            nc.vector.tensor_tensor(
                                out=kv_acc[hp], in0=kv_ps, in1=kv_acc[hp],
                                op=AL.add)

            # ---------------- kv fixup: kvT -> kvS/ksr ----------------
            # (reuses pass-A pools to stay within the 8 PSUM banks)
            for i in range(4):
                kvt_sb = kv_acc[i]
                for j in range(2):       # head h = 2i + j
                    h = 2 * i + j
                    hb = j * HD
                    ho = HD - hb         # the zero half
                    for s in range(2):   # F slab
                        tp = mmps.tile([P, HD + 1], FP32, tag="mm", name="tps")
                        nc.tensor.transpose(
                            tp, kvt_sb[:, j * F + s * P: j * F + (s + 1) * P],
                            ident[0:HD + 1, 0:HD + 1])
                        nc.scalar.copy(kvS[h][s][:, hb:hb + HD], tp[:, 0:HD])
                        # ksr[h][s][f, hb:hb+64] = k_sum[f]
                        kp2 = kpps.tile([P, P], FP32, tag="kp0", name="ksr_ps")
                        nc.tensor.matmul(
                            kp2,
                            lhsT=(kvt_sb[HD:HD + 1,
                                         j * F + s * P: j * F + (s + 1) * P]),
                            rhs=(ones_sb[HD:HD + 1, :]), start=True, stop=True)
                        nc.scalar.copy(ksr[h][s][:, hb:hb + HD],
                                       kp2[:, 0:HD])

        if dbg is not None:
            for h in range(HPC):
                for s in range(2):
                    nc.sync.dma_start(out=dbg["kvS"][h, s], in_=kvS[h][s].bitcast(FP32))
                    nc.sync.dma_start(out=dbg["ksr"][h, s], in_=ksr[h][s].bitcast(FP32))

        # ---------------- pass B ----------------
        with ExitStack() as bctx:
            wopool = bctx.enter_context(tc.tile_pool(name="wopool", bufs=1))
            wo_sb = [wopool.tile([P, D], F32R, tag=f"wo{s}", name=f"wo{s}") for s in range(4)]
            for s in range(4):
                nc.sync.dma_start(out=wo_sb[s], in_=wout[s * P:(s + 1) * P, :])

            qtbpool = bctx.enter_context(tc.tile_pool(name="qtbpool", bufs=2))
            qppool = bctx.enter_context(tc.tile_pool(name="qppool", bufs=4))
            attpool = bctx.enter_context(tc.tile_pool(name="attpool", bufs=2))
            zpool = bctx.enter_context(tc.tile_pool(name="zpool", bufs=2))
            ypool = bctx.enter_context(tc.tile_pool(name="ypool", bufs=3))
            qpps = bctx.enter_context(tc.tile_pool(name="qpps", bufs=3, space="PSUM"))
            atps = bctx.enter_context(tc.tile_pool(name="atps", bufs=2, space="PSUM"))
            dnps = bctx.enter_context(tc.tile_pool(name="dnps", bufs=1, space="PSUM"))
            yps = bctx.enter_context(tc.tile_pool(name="yps", bufs=1, space="PSUM"))

            # pair-packed qT view: pair hp -> partitions 0:64 = head 2hp,
            # 64:128 = head 2hp+1
            qTd_v = qTd.rearrange("(hh p) m -> p hh m", p=P)

            for g in range(NG):
                g0 = g * 512
                qt = qtbpool.tile([P, HPC // 2, 512], F32R, tag="qt", name="qt")
                nc.sync.dma_start(out=qt, in_=qTd_v[:, :, g0:g0 + 512])

                att_sb = [attpool.tile([P, 512], F32R, tag=f"att{i}", name=f"att{i}")
                          for i in range(4)]
                for hp in range(HPC // 2):
                    # head pair accumulates into one [128,512] PSUM tile via
                    # zero-padded lhsT (matmul out must start at partition 0):
                    # head 2hp -> partitions 0:64, head 2hp+1 -> 64:128
                    aps = atps.tile([P, 512], FP32, tag="at", name="aps")
                    dps = dnps.tile([P, 512], FP32, tag="dn", name="dn")
                    for h in (2 * hp, 2 * hp + 1):
                        hb = (h % 2) * HD
                        first = h % 2 == 0
                        last = h % 2 == 1
                        # q_projT (2 F-slabs) + elu1
                        qP = []
                        for s in range(2):
                            ps = qpps.tile([P, 512], FP32, tag="qp", name="qp")
                            nc.tensor.matmul(
                                ps, lhsT=(proj_pair[hp][hb:hb + HD,
                                                          s * P:(s + 1) * P]),
                                rhs=(qt[hb:hb + HD, hp, :]),
                                start=True, stop=True)
                            qE = qppool.tile([P, 512], FP32, tag="qE", name="qE")
                            qR = qppool.tile([P, 512], FP32, tag="qR", name="qR")
                            qPs = qppool.tile([P, 512], F32R, tag=f"qP{s}", name=f"qP{s}")
                            nc.scalar.activation(qE, ps, AF.Exp)
                            nc.scalar.activation(qR, ps, AF.Relu)
                            nc.vector.scalar_tensor_tensor(
                                qPs, in0=qE, scalar=1.0, in1=qR,
                                op0=AL.min, op1=AL.add)
                            if dbg is not None and g == 0:
                                nc.sync.dma_start(out=dbg["qp"][h, s],
                                                  in_=qPs.bitcast(FP32))
                            qP.append(qPs)

                        for s in range(2):
                            nc.tensor.matmul(
                                aps, lhsT=(kvS[h][s]), rhs=(qP[s]),
                                start=(first and s == 0),
                                stop=(last and s == 1),
                                skip_group_check=True)
                            nc.tensor.matmul(
                                dps, lhsT=(ksr[h][s]), rhs=(qP[s]),
                                start=(first and s == 0),
                                stop=(last and s == 1),
                                skip_group_check=True)
                    # z for both heads at once; evict attnT with z fused.
                    # approx reciprocal (~18 bits) is far below the fp32r
                    # matmul noise and ~5x faster than the iterative divide.
                    zb = zpool.tile([P, 512], FP32, tag="zb", name="zb")
                    nc.vector.reciprocal_approx_fast(zb, dps)
                    nc.vector.tensor_tensor(
                        out=att_sb[hp], in0=aps, in1=zb, op=AL.mult)
                    if dbg is not None and g == 0:
                        nc.sync.dma_start(out=dbg["at"][hp],
                                          in_=att_sb[hp].bitcast(FP32))
                        nc.sync.dma_start(out=dbg["dn"][hp], in_=zb)

                # y = attnT^T @ w_out; consecutive o-halves share lhsT
                for t in range(TPG):
                    pso = [yps.tile([P, 512], FP32, tag=f"y{o}", name=f"y{o}")
                           for o in range(2)]
                    for s in range(4):
                        for o in range(2):
                            nc.tensor.matmul(
                                pso[o], lhsT=(att_sb[s][:, t * P:(t + 1) * P]),
                                rhs=(wo_sb[s][:, o * 512:(o + 1) * 512]),
                                start=(s == 0), stop=(s == 3))
                    for o in range(2):
                        y_sb = ypool.tile([P, 512], FP32, tag="ysb", name="ysb")
                        nc.any.tensor_copy(y_sb, pso[o])
                        nc.scalar.dma_start(
                            out=y[g0 + t * P: g0 + (t + 1) * P,
                                  o * 512:(o + 1) * 512],
                            in_=y_sb)


def build(n=SEQ):
    # Bacc (not raw Bass): its compile pipeline splits multi-waits into
    # event semaphores (TRN2 allows at most 1 sync wait per instruction).
    nc = bacc.Bacc("TRN2", target_bir_lowering=False, debug=False,
                   enable_asserts=False)
    xT = nc.declare_dram_parameter("xT", [D, n], F32R, isOutput=False)
    wq = nc.declare_dram_parameter("wq", [D, DH], F32R, isOutput=False)
    wk = nc.declare_dram_parameter("wk", [D, DH], F32R, isOutput=False)
    wv = nc.declare_dram_parameter("wv", [D, DH], F32R, isOutput=False)
    proj = nc.declare_dram_parameter("proj", [DH, F], F32R, isOutput=False)
    wout = nc.declare_dram_parameter("wout", [DH, D], F32R, isOutput=False)
    y = nc.declare_dram_parameter("y", [n, D], FP32, isOutput=True)
    qTd = nc.dram_tensor("qT_scratch", [DH, n], F32R)
    with tile.TileContext(nc) as tc:
        _emit(tc, n, xT, wq, wk, wv, proj, wout, y, qTd)
    nc.finalize()
    return nc


def make_in_maps(x, w_qkv, proj_matrix, w_out):
    x = np.asarray(x, np.float32)
    w_qkv = np.asarray(w_qkv, np.float32)
    proj_matrix = np.asarray(proj_matrix, np.float32)
    w_out = np.asarray(w_out, np.float32)
    in_maps = []
    for c in range(NCORES):
        b, g = c // 2, c % 2
        in_maps.append({
            "xT": np.ascontiguousarray(x[b].T),
            "wq": np.ascontiguousarray(w_qkv[:, DH * g:DH * (g + 1)]),
            "wk": np.ascontiguousarray(w_qkv[:, D + DH * g:D + DH * (g + 1)]),
            "wv": np.ascontiguousarray(w_qkv[:, 2 * D + DH * g:2 * D + DH * (g + 1)]),
            "proj": np.ascontiguousarray(
                proj_matrix[HPC * g:HPC * (g + 1)].reshape(DH, F)),
            "wout": np.ascontiguousarray(w_out[DH * g:DH * (g + 1), :]),
        })
    return in_maps


_NC_CACHE = {}


def get_nc(n=SEQ):
    if n not in _NC_CACHE:
        _NC_CACHE[n] = build(n)
    return _NC_CACHE[n]


def _install_ntff_hook_shim():
    """The agent image's antenv lacks axon_hooks; recreate it so
    run_bass_kernel_spmd(trace=True) can capture NTFF profiles."""
    import sys
    import types
    try:
        from antenv.axon_hooks import get_axon_ntff_profile_hook  # noqa: F401
        return True
    except ImportError:
        pass
    try:
        from trn_agent_boot.trn_boot import _ntff_profile_via_ctypes
        import antenv
        mod = types.ModuleType("antenv.axon_hooks")
        mod._hook = _ntff_profile_via_ctypes("/opt/axon/libaxon_pjrt.so")
        mod.set_axon_ntff_profile_hook = lambda h: setattr(mod, "_hook", h)
        mod.get_axon_ntff_profile_hook = lambda: mod._hook
        sys.modules["antenv.axon_hooks"] = mod
        antenv.axon_hooks = mod
        return True
    except Exception as e:  # profiling is best-effort
        print(f"ntff hook shim failed: {e}")
        return False


def run(x, w_qkv, proj_matrix, w_out, b_out, trace=False, **kw):
    if trace:
        _install_ntff_hook_shim()
    nc = get_nc(SEQ)
    in_maps = make_in_maps(x, w_qkv, proj_matrix, w_out)
    res = run_bass_kernel_spmd(nc, in_maps, list(range(NCORES)),
                               trace=trace, **kw)
    b_out = np.asarray(b_out, np.float32)
    out = np.empty((B, SEQ, D), np.float32)
    for b in range(B):
        out[b] = res.results[2 * b]["y"] + res.results[2 * b + 1]["y"] \
            + b_out[None, :]
    return out, res


def kernel(x, w_qkv, proj_matrix, w_out, b_out):
    out, _ = run(x, w_qkv, proj_matrix, w_out, b_out)
    return out

